# revision 8
# baseline (speedup 1.0000x reference)
"""Trainium2 Bass kernel for nn_Block_9981503996445 (dense_transformer).

Block: GroupNorm(1,256) -> 3x3 conv 256->384 -> split q/k/v/u ->
depthwise stride-2 downsample of k,v -> attention softmax(q^T k / 8) over
M=1024 -> a = v @ attn^T -> concat(a,u) -> relu -> 1x1 conv + residual ->
FFN (1x1 -> relu -> 1x1) + residual.

Sharding: data-parallel over batch, B=16 -> 2 samples per core on 8 cores.
All matmuls run in float32r (TF32-like, ~1.6e-4 relerr, 1 cyc/row at N=512).

Key algebraic simplifications (host-side folds):
 - All BatchNorm scales folded into conv weights; biases folded into PSUM
   drains (DVE tensor_scalar) or relu bias (ACT).
 - b_k dropped entirely: it adds a per-n constant to softmax logits over m,
   which softmax is invariant to.
 - b_v applied after attention normalization: sum_m attn = 1, so the bias
   passes through the attention average unchanged -> folded into relu bias.
 - Softmax computed without max subtraction: logits*scale lie in [-1, 1].
Attention is computed in transposed layout St[m, n] so the contraction of
a = vf @ expSt happens directly on the PE without transposing the [N, M]
attention matrix; the denominator is reduced over partitions via a DVE
add-tree + ones-vector matmul, and its reciprocal is broadcast back across
partitions with a K=1 ones matmul.
"""

import sys

for _p in ("/opt/trn_rl_repo",):
    if _p not in sys.path:
        sys.path.insert(0, _p)

from contextlib import ExitStack

import numpy as np

import concourse.bacc as bacc
import concourse.mybir as mybir
import concourse.tile as tile
from concourse.bass_utils import run_bass_kernel_spmd

F32 = mybir.dt.float32
F32R = mybir.dt.float32r
AF = mybir.ActivationFunctionType
OP = mybir.AluOpType

P = 128
DIM, QK, PD, HID = 256, 64, 128, 512
H = W = 64
N = H * W            # 4096
M = 1024             # 32*32 after stride-2 downsample
WP = W + 2           # 66, padded row stride
NPAD = WP * (H + 2)  # 4356
SCALE = QK ** -0.5
EPS = 1e-5
NCORES = 8
SPC = 2              # samples per core
NBLK = 8             # n blocks per sample
BW = N // NBLK       # 512, block width
RB = BW // W         # 8 rows per block

_CACHE = {}


def _prep_weights(w_in, s_in, b_in, w_k, s_k, w_v, s_v, b_v,
                  w_out, s_out, b_out, w1, s1, b1, w2, s2, b2, gn_w, gn_b):
    """Fold BN scales into weights; build lhsT tile layouts.

    t channel layout (3 tiles x 128 partitions):
      tile0: [u[0:64] ; q]   tile1: [u[64:128] ; k]   tile2: v
    so that q and k both sit at base partition 64 (matmul base match), and
    v at base 0.
    """
    wf = (w_in * s_in[:, None, None, None]).astype(np.float32)  # [384,256,3,3]
    # reference t channels: q=0:64, k=64:128, v=128:256, u=256:384
    refch = np.empty((3, P), np.int64)
    refch[0, :64] = np.arange(256, 320)   # u[0:64]
    refch[0, 64:] = np.arange(0, 64)      # q
    refch[1, :64] = np.arange(320, 384)   # u[64:128]
    refch[1, 64:] = np.arange(64, 128)    # k
    refch[2, :] = np.arange(128, 256)     # v

    WIN = np.empty((9, 2, 3, P, P), np.float32)  # [tap, ct, ot, c, o]
    for ky in range(3):
        for kx in range(3):
            tap = ky * 3 + kx
            for ct in range(2):
                for ot in range(3):
                    # lhsT[c, o] = wf[refch(ot,o), ct*128+c, ky, kx]
                    WIN[tap, ct, ot] = wf[refch[ot], ct * P:(ct + 1) * P, ky, kx].T
    BIN = b_in[refch].reshape(3, P, 1).astype(np.float32)

    # depthwise downsample weights [dy,dx] taps; k lives at partitions 64:128
    WK = np.zeros((P, 4), np.float32)
    WK[64:, :] = (w_k[:, 0] * s_k[:, None, None]).reshape(QK, 4)
    WV = (w_v[:, 0] * s_v[:, None, None]).reshape(PD, 4).astype(np.float32)
    BV = b_v.reshape(PD, 1).astype(np.float32)

    wo = (w_out[:, :, 0, 0] * s_out[:, None]).astype(np.float32)  # [256,256]
    # cat channels: 0:128 = a (PD), 128:256 = u
    WOUTA = np.stack([wo[ot * P:(ot + 1) * P, 0:128].T for ot in range(2)])
    WOUTU0 = np.stack([wo[ot * P:(ot + 1) * P, 128:192].T for ot in range(2)])
    WOUTU1 = np.stack([wo[ot * P:(ot + 1) * P, 192:256].T for ot in range(2)])
    BOUT = b_out.reshape(2, P, 1).astype(np.float32)

    w1f = (w1[:, :, 0, 0] * s1[:, None]).astype(np.float32)  # [512,256]
    W1T = np.empty((2, 4, P, P), np.float32)
    for kt in range(2):
        for ot in range(4):
            W1T[kt, ot] = w1f[ot * P:(ot + 1) * P, kt * P:(kt + 1) * P].T
    B1 = b1.reshape(4, P, 1).astype(np.float32)

    w2f = (w2[:, :, 0, 0] * s2[:, None]).astype(np.float32)  # [256,512]
    W2T = np.empty((4, 2, P, P), np.float32)
    for kt in range(4):
        for ot in range(2):
            W2T[kt, ot] = w2f[ot * P:(ot + 1) * P, kt * P:(kt + 1) * P].T
    B2 = b2.reshape(2, P, 1).astype(np.float32)

    GNW = gn_w.reshape(2, P, 1).astype(np.float32)
    GNB = gn_b.reshape(2, P, 1).astype(np.float32)

    return dict(
        win=WIN.reshape(54, P, P), bin=BIN, wk=WK, wv=WV, bv=BV,
        wouta=WOUTA, woutu0=WOUTU0, woutu1=WOUTU1, bout=BOUT,
        w1t=W1T, b1=B1, w2t=W2T, b2=B2, gnw=GNW, gnb=GNB,
    )


def _build(n_cores):
    nc = bacc.Bacc("TRN2", target_bir_lowering=False, debug=False,
                   num_devices=n_cores)

    x_d = nc.dram_tensor("x", [SPC, DIM, N], F32, kind="ExternalInput").ap()
    win_d = nc.dram_tensor("win", [54, P, P], F32, kind="ExternalInput").ap()
    bin_d = nc.dram_tensor("bin", [3, P, 1], F32, kind="ExternalInput").ap()
    wk_d = nc.dram_tensor("wk", [P, 4], F32, kind="ExternalInput").ap()
    wv_d = nc.dram_tensor("wv", [P, 4], F32, kind="ExternalInput").ap()
    bv_d = nc.dram_tensor("bv", [P, 1], F32, kind="ExternalInput").ap()
    wouta_d = nc.dram_tensor("wouta", [2, P, P], F32, kind="ExternalInput").ap()
    woutu0_d = nc.dram_tensor("woutu0", [2, 64, P], F32, kind="ExternalInput").ap()
    woutu1_d = nc.dram_tensor("woutu1", [2, 64, P], F32, kind="ExternalInput").ap()
    bout_d = nc.dram_tensor("bout", [2, P, 1], F32, kind="ExternalInput").ap()
    w1t_d = nc.dram_tensor("w1t", [2, 4, P, P], F32, kind="ExternalInput").ap()
    b1_d = nc.dram_tensor("b1", [4, P, 1], F32, kind="ExternalInput").ap()
    w2t_d = nc.dram_tensor("w2t", [4, 2, P, P], F32, kind="ExternalInput").ap()
    b2_d = nc.dram_tensor("b2", [2, P, 1], F32, kind="ExternalInput").ap()
    gnw_d = nc.dram_tensor("gnw", [2, P, 1], F32, kind="ExternalInput").ap()
    gnb_d = nc.dram_tensor("gnb", [2, P, 1], F32, kind="ExternalInput").ap()
    out_d = nc.dram_tensor("out", [SPC, DIM, N], F32, kind="ExternalOutput").ap()

    xv_d = x_d.rearrange("s (ct p) n -> s ct p n", p=P)
    ov_d = out_d.rearrange("s (ct p) n -> s ct p n", p=P)

    with tile.TileContext(nc) as tc, ExitStack() as ctx, \
            nc.allow_low_precision(reason="float32r matmul inputs"):
        consts = ctx.enter_context(tc.tile_pool(name="consts", bufs=1))
        wp = ctx.enter_context(tc.tile_pool(name="wp", bufs=1))
        stage = ctx.enter_context(tc.tile_pool(name="stage", bufs=2))
        xin = ctx.enter_context(tc.tile_pool(name="xin", bufs=3))
        scratch = ctx.enter_context(tc.tile_pool(name="scratch", bufs=2))
        tp = ctx.enter_context(tc.tile_pool(name="tp", bufs=1))
        kvp = ctx.enter_context(tc.tile_pool(name="kvp", bufs=1))
        small = ctx.enter_context(tc.tile_pool(name="small", bufs=2))
        blk = ctx.enter_context(tc.tile_pool(name="blk", bufs=1))
        blk2 = ctx.enter_context(tc.tile_pool(name="blk2", bufs=2))
        ps = ctx.enter_context(tc.tile_pool(name="ps", bufs=1, space="PSUM"))
        ps2 = ctx.enter_context(tc.tile_pool(name="ps2", bufs=2, space="PSUM"))

        # 8 PSUM banks total; pools reserve statically per tag:
        #   mm0 x2 (conv tps / attn st), a x2, acc x2 (wout/ffn), dn x1, bc x1
        _PSUM_MAP = {"tps": ("mm0", ps2), "st": ("mm0", ps2),
                     "a": ("a", ps2), "o": ("acc", ps2), "h": ("acc", ps2),
                     "f": ("acc", ps2), "dn": ("dn", ps), "gn1": ("dn", ps),
                     "gn2": ("dn", ps), "tpose": ("dn", ps), "bc": ("bc", ps)}

        def psum(shape, tag, dtype=F32):
            tag, pool = _PSUM_MAP[tag]
            return pool.tile(shape, dtype, tag=tag, name=tag, space="PSUM")

        # ---- constants ----
        ones_f = consts.tile([P, 1], F32, tag="ones_f", name="ones_f")
        nc.vector.memset(ones_f, 1.0)
        ones_r = consts.tile([P, 1], F32R, tag="ones_r", name="ones_r")
        nc.vector.tensor_copy(out=ones_r, in_=ones_f)
        ones1_f = consts.tile([1, P], F32, tag="ones1_f", name="ones1_f")
        nc.vector.memset(ones1_f, 1.0)
        ones1_r = consts.tile([1, P], F32R, tag="ones1_r", name="ones1_r")
        nc.vector.tensor_copy(out=ones1_r, in_=ones1_f)
        eps_t = consts.tile([1, 1], F32, tag="eps", name="eps")
        nc.vector.memset(eps_t, EPS)
        ident_f = consts.tile([P, P], F32, tag="ident_f", name="ident_f")
        from concourse.masks import make_identity
        make_identity(nc, ident_f)
        ident_r = consts.tile([P, P], F32R, tag="ident_r", name="ident_r")
        nc.vector.tensor_copy(out=ident_r, in_=ident_f)

        # ---- weights: DMA f32, round to f32r via DVE copies ----
        def load_r(dram_ap, tag):
            st = stage.tile(list(dram_ap.shape), F32, tag="stage", name="stage")
            nc.sync.dma_start(out=st, in_=dram_ap)
            wt = wp.tile(list(dram_ap.shape), F32R, tag=tag, name=tag)
            nc.vector.tensor_copy(out=wt, in_=st)
            return wt

        def load_f(dram_ap, tag):
            t = wp.tile(list(dram_ap.shape), F32, tag=tag, name=tag)
            nc.sync.dma_start(out=t, in_=dram_ap)
            return t

        win_sb = [load_r(win_d[i], f"win{i}") for i in range(54)]
        wouta_sb = [load_r(wouta_d[i], f"wouta{i}") for i in range(2)]
        woutu0_sb = [load_r(woutu0_d[i], f"woutu0{i}") for i in range(2)]
        woutu1_sb = [load_r(woutu1_d[i], f"woutu1{i}") for i in range(2)]
        w1t_sb = [[load_r(w1t_d[kt, ot], f"w1t{kt}{ot}") for ot in range(4)]
                  for kt in range(2)]
        w2t_sb = [[load_r(w2t_d[kt, ot], f"w2t{kt}{ot}") for ot in range(2)]
                  for kt in range(4)]
        bin_sb = [load_f(bin_d[i], f"bin{i}") for i in range(3)]
        bout_sb = [load_f(bout_d[i], f"bout{i}") for i in range(2)]
        b1_sb = [load_f(b1_d[i], f"b1_{i}") for i in range(4)]
        b2_sb = [load_f(b2_d[i], f"b2_{i}") for i in range(2)]
        bv_sb = load_f(bv_d, "bv")
        wk_sb = load_f(wk_d, "wk")
        wv_sb = load_f(wv_d, "wv")
        gnw_sb = [load_f(gnw_d[i], f"gnw{i}") for i in range(2)]
        gnb_sb = [load_f(gnb_d[i], f"gnb{i}") for i in range(2)]

        for s in range(SPC):
            _build_sample(nc, tc, s, xv_d, ov_d, psum,
                          xin, scratch, tp, kvp, small, blk, blk2,
                          win_sb, bin_sb, wk_sb, wv_sb, bv_sb,
                          wouta_sb, woutu0_sb, woutu1_sb, bout_sb,
                          w1t_sb, b1_sb, w2t_sb, b2_sb, gnw_sb, gnb_sb,
                          ones_f, ones_r, ones1_f, ones1_r, eps_t, ident_r)

    nc.compile()
    return nc


def _build_sample(nc, tc, s, xv_d, ov_d, psum,
                  xin, scratch, tp, kvp, small, blk, blk2,
                  win_sb, bin_sb, wk_sb, wv_sb, bv_sb,
                  wouta_sb, woutu0_sb, woutu1_sb, bout_sb,
                  w1t_sb, b1_sb, w2t_sb, b2_sb, gnw_sb, gnb_sb,
                  ones_f, ones_r, ones1_f, ones1_r, eps_t, ident_r):
    # ============ P1: GroupNorm stats (stream x, pass 1) ============
    stats = [small.tile([P, NBLK, 6], F32, tag=f"stats{ct}", name=f"stats{ct}") for ct in range(2)]
    for ct in range(2):
        for b in range(NBLK):
            xc = xin.tile([P, BW], F32, tag="xc", name="xc")
            nc.sync.dma_start(out=xc, in_=xv_d[s, ct, :, b * BW:(b + 1) * BW])
            nc.vector.bn_stats(out=stats[ct][:, b, :], in_=xc)
    packed = small.tile([P, 4], F32, tag="packed", name="packed")
    for ct in range(2):
        mv = small.tile([P, 2], F32, tag=f"mv{ct}", name=f"mv{ct}")
        nc.vector.bn_aggr(out=mv, in_=stats[ct])
        # packed cols: [mean_ct, E[x^2]_ct]
        nc.vector.tensor_copy(out=packed[:, 2 * ct:2 * ct + 1], in_=mv[:, 0:1])
        m2 = small.tile([P, 1], F32, tag=f"m2{ct}", name=f"m2{ct}")
        nc.vector.tensor_mul(out=m2, in0=mv[:, 0:1], in1=mv[:, 0:1])
        nc.vector.tensor_add(out=packed[:, 2 * ct + 1:2 * ct + 2],
                             in0=mv[:, 1:2], in1=m2)
    gsum = psum([1, 4], "gn1")
    nc.tensor.matmul(gsum, ones_f, packed, start=True, stop=True)
    # scalars on partition 0
    gs = small.tile([1, 4], F32, tag="gs", name="gs")
    nc.vector.tensor_copy(out=gs, in_=gsum)
    sc = small.tile([1, 4], F32, tag="sc", name="sc")  # [mu, e2, var, sd]
    nc.vector.tensor_add(out=sc[:, 0:1], in0=gs[:, 0:1], in1=gs[:, 2:3])
    nc.scalar.mul(out=sc[:, 0:1], in_=sc[:, 0:1], mul=1.0 / DIM)
    nc.vector.tensor_add(out=sc[:, 1:2], in0=gs[:, 1:2], in1=gs[:, 3:4])
    nc.scalar.mul(out=sc[:, 1:2], in_=sc[:, 1:2], mul=1.0 / DIM)
    mu2 = small.tile([1, 1], F32, tag="mu2", name="mu2")
    nc.vector.tensor_mul(out=mu2, in0=sc[:, 0:1], in1=sc[:, 0:1])
    nc.vector.tensor_tensor(out=sc[:, 2:3], in0=sc[:, 1:2], in1=mu2,
                            op=OP.subtract)
    nc.scalar.activation(out=sc[:, 3:4], in_=sc[:, 2:3], func=AF.Sqrt,
                         bias=eps_t, scale=1.0)
    rv = small.tile([1, 2], F32, tag="rv", name="rv")  # [mu, rstd]
    nc.vector.tensor_copy(out=rv[:, 0:1], in_=sc[:, 0:1])
    nc.vector.reciprocal(out=rv[:, 1:2], in_=sc[:, 3:4])
    gbc = psum([P, 2], "gn2")
    nc.tensor.matmul(gbc, ones1_f, rv, start=True, stop=True)  # bcast [128,2]
    A = [small.tile([P, 1], F32, tag=f"A{ct}", name=f"A{ct}") for ct in range(2)]
    B = [small.tile([P, 1], F32, tag=f"B{ct}", name=f"B{ct}") for ct in range(2)]
    for ct in range(2):
        nc.vector.tensor_mul(out=A[ct], in0=gnw_sb[ct], in1=gbc[:, 1:2])
        tmp = small.tile([P, 1], F32, tag=f"ab{ct}", name=f"ab{ct}")
        nc.vector.tensor_mul(out=tmp, in0=A[ct], in1=gbc[:, 0:1])
        nc.vector.tensor_tensor(out=B[ct], in0=gnb_sb[ct], in1=tmp,
                                op=OP.subtract)

    # ============ P2: xn_pad (stream x, pass 2) + 3x3 conv ============
    xn = [scratch.tile([P, NPAD], F32R, tag="big", name="big") for _ in range(2)]
    xnv = [t.rearrange("p (h w) -> p h w", w=WP) for t in xn]
    for ct in range(2):
        v = xnv[ct]
        nc.vector.memset(v[:, 0, :].bitcast(F32), 0.0)
        nc.vector.memset(v[:, H + 1, :].bitcast(F32), 0.0)
        nc.vector.memset(v[:, 1:H + 1, 0:1].bitcast(F32), 0.0)
        nc.vector.memset(v[:, 1:H + 1, WP - 1:WP].bitcast(F32), 0.0)
        for b in range(NBLK):
            xc = xin.tile([P, BW], F32, tag="xc", name="xc")
            nc.sync.dma_start(out=xc, in_=xv_d[s, ct, :, b * BW:(b + 1) * BW])
            nc.vector.tensor_scalar(
                out=v[:, 1 + RB * b:1 + RB * (b + 1), 1:W + 1],
                in0=xc.rearrange("p (h w) -> p h w", w=W),
                scalar1=A[ct], scalar2=B[ct], op0=OP.mult, op1=OP.add)

    t_sb = [tp.tile([P, N], F32R, tag=f"t{ot}", name=f"t{ot}") for ot in range(3)]
    for ot in range(3):
        for b in range(NBLK):
            y0 = RB * b
            pt = psum([P, BW], "tps")
            first = True
            for ct in range(2):
                for ky in range(3):
                    for kx in range(3):
                        last = (ct == 1 and ky == 2 and kx == 2)
                        nc.tensor.matmul(
                            pt, win_sb[(ky * 3 + kx) * 2 * 3 + ct * 3 + ot],
                            xnv[ct][:, y0 + ky:y0 + ky + RB, kx:kx + W],
                            start=first, stop=last)
                        first = False
            nc.vector.tensor_scalar(
                out=t_sb[ot][:, y0 * W:(y0 + RB) * W], in0=pt,
                scalar1=bin_sb[ot], scalar2=None, op0=OP.add)

    # ============ P3: k/v depthwise stride-2 downsample, vfT ============
    kf = kvp.tile([P, M], F32R, tag="kf", name="kf")   # valid on partitions 64:128
    vf = kvp.tile([P, M], F32R, tag="vf", name="vf")
    for (dst, dview, src, wsc, p0) in (
            (kf, kf[64:, :].rearrange("p (h w) -> p h w", w=32),
             t_sb[1][64:, :].rearrange("p (h w) -> p h w", w=W), wk_sb, 64),
            (vf, vf.rearrange("p (h w) -> p h w", w=32),
             t_sb[2].rearrange("p (h w) -> p h w", w=W), wv_sb, 0)):
        for j, (dy, dx) in enumerate(((0, 0), (0, 1), (1, 0), (1, 1))):
            sj = src[:, dy::2, dx::2]
            if j == 0:
                nc.vector.tensor_scalar(out=dview, in0=sj,
                                        scalar1=wsc[p0:, 0:1], scalar2=None,
                                        op0=OP.mult)
            else:
                nc.vector.scalar_tensor_tensor(
                    out=dview, in0=sj, scalar=wsc[p0:, j:j + 1], in1=dview,
                    op0=OP.mult, op1=OP.add)

    vfT = []
    for mt in range(8):
        ptr = psum([P, P], "tpose", dtype=F32R)
        nc.tensor.transpose(ptr, vf[:, mt * P:(mt + 1) * P], ident_r)
        vt = kvp.tile([P, P], F32R, tag=f"vfT{mt}", name=f"vfT{mt}")
        nc.vector.tensor_copy(out=vt, in_=ptr)
        vfT.append(vt)

    # ============ P4: attention + FFN, 8 blocks of 512 ============
    qf = t_sb[0][64:, :]
    for b in range(NBLK):
        n0 = b * BW
        est = scratch.tile([P, 8, BW], F32R, tag="big", name="big")  # exp(St) supertile
        for mt in range(8):
            st_ps = psum([P, BW], "st")
            nc.tensor.matmul(st_ps, kf[64:, mt * P:(mt + 1) * P],
                             qf[:, n0:n0 + BW], start=True, stop=True)
            nc.scalar.activation(out=est[:, mt, :], in_=st_ps, func=AF.Exp,
                                 scale=SCALE)
        # a_raw = vf @ expSt  (contraction over m)
        a_ps = psum([P, BW], "a")
        for mt in range(8):
            nc.tensor.matmul(a_ps, vfT[mt], est[:, mt, :],
                             start=(mt == 0), stop=(mt == 7))
        # denominator: sum over m = partitions of est tiles
        part = blk.tile([P, BW], F32R, tag="part", name="part")
        nc.vector.tensor_add(out=part, in0=est[:, 0, :], in1=est[:, 1, :])
        for mt in range(2, 8):
            nc.vector.tensor_add(out=part, in0=part, in1=est[:, mt, :])
        dn_ps = psum([1, BW], "dn")
        nc.tensor.matmul(dn_ps, ones_r, part, start=True, stop=True)
        rec = blk.tile([1, BW], F32R, tag="rec", name="rec")
        nc.vector.reciprocal(out=rec, in_=dn_ps)
        bc_ps = psum([P, BW], "bc")
        nc.tensor.matmul(bc_ps, ones1_r, rec, start=True, stop=True)
        bc = blk.tile([P, BW], F32, tag="bc", name="bc")
        nc.vector.tensor_copy(out=bc, in_=bc_ps)
        asc = blk.tile([P, BW], F32, tag="asc", name="asc")
        nc.vector.tensor_mul(out=asc, in0=a_ps, in1=bc)
        ra = blk.tile([P, BW], F32R, tag="ra", name="ra")
        nc.scalar.activation(out=ra, in_=asc, func=AF.Relu, bias=bv_sb,
                             scale=1.0)
        ru = [blk.tile([64, BW], F32R, tag=f"ru{i}", name=f"ru{i}") for i in range(2)]
        for i in range(2):
            nc.scalar.activation(out=ru[i], in_=t_sb[i][0:64, n0:n0 + BW],
                                 func=AF.Relu)
        # wout 1x1 + bias + residual -> x1
        x1 = []
        for ot in range(2):
            o_ps = psum([P, BW], "o")
            nc.tensor.matmul(o_ps, wouta_sb[ot], ra, start=True, stop=False)
            nc.tensor.matmul(o_ps, woutu0_sb[ot], ru[0], start=False, stop=False)
            nc.tensor.matmul(o_ps, woutu1_sb[ot], ru[1], start=False, stop=True)
            xr = xin.tile([P, BW], F32, tag="xres", name="xres")
            nc.sync.dma_start(out=xr, in_=xv_d[s, ot, :, n0:n0 + BW])
            x1t = blk2.tile([P, BW], F32R, tag=f"x1_{ot}", name=f"x1_{ot}")
            nc.vector.scalar_tensor_tensor(out=x1t, in0=o_ps,
                                           scalar=bout_sb[ot], in1=xr,
                                           op0=OP.add, op1=OP.add)
            x1.append(x1t)
        # FFN
        hs = []
        for ot in range(4):
            h_ps = psum([P, BW], "h")
            for kt in range(2):
                nc.tensor.matmul(h_ps, w1t_sb[kt][ot], x1[kt],
                                 start=(kt == 0), stop=(kt == 1))
            ht = blk.tile([P, BW], F32R, tag=f"h{ot}", name=f"h{ot}")
            nc.scalar.activation(out=ht, in_=h_ps, func=AF.Relu,
                                 bias=b1_sb[ot], scale=1.0)
            hs.append(ht)
        for ot in range(2):
            f_ps = psum([P, BW], "f")
            for kt in range(4):
                nc.tensor.matmul(f_ps, w2t_sb[kt][ot], hs[kt],
                                 start=(kt == 0), stop=(kt == 3))
            ob = blk2.tile([P, BW], F32, tag=f"ob{ot}", name=f"ob{ot}")
            nc.vector.scalar_tensor_tensor(out=ob, in0=f_ps,
                                           scalar=b2_sb[ot], in1=x1[ot],
                                           op0=OP.add, op1=OP.add)
            nc.sync.dma_start(out=ov_d[s, ot, :, n0:n0 + BW], in_=ob)


def kernel(**inputs):
    x = np.ascontiguousarray(np.asarray(inputs["x"], dtype=np.float32))
    B = x.shape[0]
    assert B == NCORES * SPC
    w = _prep_weights(
        inputs["w_in"], inputs["s_in"], inputs["b_in"],
        inputs["w_k"], inputs["s_k"], inputs["w_v"], inputs["s_v"],
        inputs["b_v"], inputs["w_out"], inputs["s_out"], inputs["b_out"],
        inputs["w1"], inputs["s1"], inputs["b1"],
        inputs["w2"], inputs["s2"], inputs["b2"],
        inputs["gn_w"], inputs["gn_b"])
    w = {k: np.ascontiguousarray(v) for k, v in w.items()}

    if "nc" not in _CACHE:
        _CACHE["nc"] = _build(NCORES)
    nc = _CACHE["nc"]

    in_maps = []
    for c in range(NCORES):
        m = dict(w)
        m["x"] = np.ascontiguousarray(
            x[c * SPC:(c + 1) * SPC].reshape(SPC, DIM, N))
        in_maps.append(m)

    res = run_bass_kernel_spmd(nc, in_maps, list(range(NCORES)))
    _CACHE["last_result"] = res
    out = np.concatenate([r["out"] for r in res.results], axis=0)
    return out.reshape(B, DIM, H, W).astype(np.float32)


if __name__ == "__main__":
    rng = np.random.default_rng(0)
    # smoke test with random weights only (no reference available here)
    print("building...")
    nc = _build(NCORES)
    print("built ok")


# revision 15
# speedup vs baseline: 8723.2158x; 8723.2158x over previous
"""Trainium2 Bass kernel for nn_Block_9981503996445 (dense_transformer).

Block: GroupNorm(1,256) -> 3x3 conv 256->384 -> split q/k/v/u ->
depthwise stride-2 downsample of k,v -> attention softmax(q^T k / 8) over
M=1024 -> a = v @ attn^T -> concat(a,u) -> relu -> 1x1 conv + residual ->
FFN (1x1 -> relu -> 1x1) + residual.

Sharding: data-parallel over batch, B=16 -> 2 samples per core on 8 cores.
All matmuls run in float32r (TF32-like, ~1.6e-4 relerr, 1 cyc/row at N=512).

Key algebraic simplifications (host-side folds):
 - All BatchNorm scales folded into conv weights; biases folded into PSUM
   drains (DVE tensor_scalar) or relu bias (ACT).
 - b_k dropped entirely: it adds a per-n constant to softmax logits over m,
   which softmax is invariant to.
 - b_v applied after attention normalization: sum_m attn = 1, so the bias
   passes through the attention average unchanged -> folded into relu bias.
 - Softmax computed without max subtraction: logits*scale lie in [-1, 1].
Attention is computed in transposed layout St[m, n] so the contraction of
a = vf @ expSt happens directly on the PE without transposing the [N, M]
attention matrix; the denominator is reduced over partitions via a DVE
add-tree + ones-vector matmul, and its reciprocal is broadcast back across
partitions with a K=1 ones matmul.
"""

import sys

for _p in ("/opt/trn_rl_repo",):
    if _p not in sys.path:
        sys.path.insert(0, _p)

from contextlib import ExitStack

import numpy as np

import concourse.bacc as bacc
import concourse.mybir as mybir
import concourse.tile as tile
from concourse.bass_utils import run_bass_kernel_spmd

F32 = mybir.dt.float32
F32R = mybir.dt.float32r
BF16 = mybir.dt.bfloat16
import os as _os
# 0: float32r everywhere; 1: conv in bf16; 2: all matmuls bf16
KBF = int(_os.environ.get("KBF", "0"))
CDT = BF16 if KBF >= 1 else F32R   # conv dtype (xn_pad, win)
ADT = BF16 if KBF >= 2 else F32R   # attention/ffn matmul dtype
AF = mybir.ActivationFunctionType
OP = mybir.AluOpType

P = 128
DIM, QK, PD, HID = 256, 64, 128, 512
H = W = 64
N = H * W            # 4096
M = 1024             # 32*32 after stride-2 downsample
WP = W + 2           # 66, padded row stride
NPAD = WP * (H + 2)  # 4356
SCALE = QK ** -0.5
EPS = 1e-5
NCORES = 8
SPC = 2              # samples per core
NBLK = 8             # n blocks per sample
BW = N // NBLK       # 512, block width
RB = BW // W         # 8 rows per block

_CACHE = {}


def _prep_weights(w_in, s_in, b_in, w_k, s_k, w_v, s_v, b_v,
                  w_out, s_out, b_out, w1, s1, b1, w2, s2, b2, gn_w, gn_b):
    """Fold BN scales into weights; build lhsT tile layouts.

    t channel layout (3 tiles x 128 partitions):
      tile0: [u[0:64] ; q]   tile1: [u[64:128] ; k]   tile2: v
    so that q and k both sit at base partition 64 (matmul base match), and
    v at base 0.
    """
    wf = (w_in * s_in[:, None, None, None]).astype(np.float32)  # [384,256,3,3]
    # reference t channels: q=0:64, k=64:128, v=128:256, u=256:384
    refch = np.empty((3, P), np.int64)
    refch[0, :64] = np.arange(256, 320)   # u[0:64]
    refch[0, 64:] = np.arange(0, 64)      # q
    refch[1, :64] = np.arange(320, 384)   # u[64:128]
    refch[1, 64:] = np.arange(64, 128)    # k
    refch[2, :] = np.arange(128, 256)     # v

    WIN = np.empty((9, 2, 3, P, P), np.float32)  # [tap, ct, ot, c, o]
    for ky in range(3):
        for kx in range(3):
            tap = ky * 3 + kx
            for ct in range(2):
                for ot in range(3):
                    # lhsT[c, o] = wf[refch(ot,o), ct*128+c, ky, kx]
                    WIN[tap, ct, ot] = wf[refch[ot], ct * P:(ct + 1) * P, ky, kx].T
    BIN = b_in[refch].reshape(3, P, 1).astype(np.float32)

    # depthwise downsample weights [dy,dx] taps; k lives at partitions 64:128
    WK = np.zeros((P, 4), np.float32)
    WK[64:, :] = (w_k[:, 0] * s_k[:, None, None]).reshape(QK, 4)
    WV = (w_v[:, 0] * s_v[:, None, None]).reshape(PD, 4).astype(np.float32)
    BV = b_v.reshape(PD, 1).astype(np.float32)

    wo = (w_out[:, :, 0, 0] * s_out[:, None]).astype(np.float32)  # [256,256]
    # cat channels: 0:128 = a (PD), 128:256 = u
    WOUTA = np.stack([wo[ot * P:(ot + 1) * P, 0:128].T for ot in range(2)])
    WOUTU0 = np.stack([wo[ot * P:(ot + 1) * P, 128:192].T for ot in range(2)])
    WOUTU1 = np.stack([wo[ot * P:(ot + 1) * P, 192:256].T for ot in range(2)])
    BOUT = b_out.reshape(2, P, 1).astype(np.float32)

    w1f = (w1[:, :, 0, 0] * s1[:, None]).astype(np.float32)  # [512,256]
    W1T = np.empty((2, 4, P, P), np.float32)
    for kt in range(2):
        for ot in range(4):
            W1T[kt, ot] = w1f[ot * P:(ot + 1) * P, kt * P:(kt + 1) * P].T
    B1 = b1.reshape(4, P, 1).astype(np.float32)

    w2f = (w2[:, :, 0, 0] * s2[:, None]).astype(np.float32)  # [256,512]
    W2T = np.empty((4, 2, P, P), np.float32)
    for kt in range(4):
        for ot in range(2):
            W2T[kt, ot] = w2f[ot * P:(ot + 1) * P, kt * P:(kt + 1) * P].T
    B2 = b2.reshape(2, P, 1).astype(np.float32)

    GNW = gn_w.reshape(2, P, 1).astype(np.float32)
    GNB = gn_b.reshape(2, P, 1).astype(np.float32)

    return dict(
        win=WIN.reshape(54, P, P), bin=BIN, wk=WK, wv=WV, bv=BV,
        wouta=WOUTA, woutu0=WOUTU0, woutu1=WOUTU1, bout=BOUT,
        w1t=W1T, b1=B1, w2t=W2T, b2=B2, gnw=GNW, gnb=GNB,
    )


def _build(n_cores, nrep=1):
    nc = bacc.Bacc("TRN2", target_bir_lowering=False, debug=False,
                   num_devices=n_cores)

    x_d = nc.dram_tensor("x", [SPC, DIM, N], F32, kind="ExternalInput").ap()
    win_d = nc.dram_tensor("win", [54, P, P], F32, kind="ExternalInput").ap()
    bin_d = nc.dram_tensor("bin", [3, P, 1], F32, kind="ExternalInput").ap()
    wk_d = nc.dram_tensor("wk", [P, 4], F32, kind="ExternalInput").ap()
    wv_d = nc.dram_tensor("wv", [P, 4], F32, kind="ExternalInput").ap()
    bv_d = nc.dram_tensor("bv", [P, 1], F32, kind="ExternalInput").ap()
    wouta_d = nc.dram_tensor("wouta", [2, P, P], F32, kind="ExternalInput").ap()
    woutu0_d = nc.dram_tensor("woutu0", [2, 64, P], F32, kind="ExternalInput").ap()
    woutu1_d = nc.dram_tensor("woutu1", [2, 64, P], F32, kind="ExternalInput").ap()
    bout_d = nc.dram_tensor("bout", [2, P, 1], F32, kind="ExternalInput").ap()
    w1t_d = nc.dram_tensor("w1t", [2, 4, P, P], F32, kind="ExternalInput").ap()
    b1_d = nc.dram_tensor("b1", [4, P, 1], F32, kind="ExternalInput").ap()
    w2t_d = nc.dram_tensor("w2t", [4, 2, P, P], F32, kind="ExternalInput").ap()
    b2_d = nc.dram_tensor("b2", [2, P, 1], F32, kind="ExternalInput").ap()
    gnw_d = nc.dram_tensor("gnw", [2, P, 1], F32, kind="ExternalInput").ap()
    gnb_d = nc.dram_tensor("gnb", [2, P, 1], F32, kind="ExternalInput").ap()
    out_d = nc.dram_tensor("out", [SPC, DIM, N], F32, kind="ExternalOutput").ap()

    xv_d = x_d.rearrange("s (ct p) n -> s ct p n", p=P)
    ov_d = out_d.rearrange("s (ct p) n -> s ct p n", p=P)

    with tile.TileContext(nc) as tc, ExitStack() as ctx, \
            nc.allow_low_precision(reason="float32r matmul inputs"):
        consts = ctx.enter_context(tc.tile_pool(name="consts", bufs=1))
        wp = ctx.enter_context(tc.tile_pool(name="wp", bufs=1))
        stage = ctx.enter_context(tc.tile_pool(name="stage", bufs=2))  # weights staging
        xin = ctx.enter_context(tc.tile_pool(name="xin", bufs=2))
        scratch = ctx.enter_context(tc.tile_pool(name="scratch", bufs=3))
        tp = ctx.enter_context(tc.tile_pool(name="tp", bufs=1))
        kvp = ctx.enter_context(tc.tile_pool(name="kvp", bufs=1))
        small = ctx.enter_context(tc.tile_pool(name="small", bufs=2))
        blk = ctx.enter_context(tc.tile_pool(name="blk", bufs=1))
        blk2 = ctx.enter_context(tc.tile_pool(name="blk2", bufs=2))
        ps = ctx.enter_context(tc.tile_pool(name="ps", bufs=1, space="PSUM"))
        ps2 = ctx.enter_context(tc.tile_pool(name="ps2", bufs=2, space="PSUM"))
        ps3 = ctx.enter_context(tc.tile_pool(name="ps3", bufs=3, space="PSUM"))

        # 8 PSUM banks total; pools reserve statically per tag:
        #   mm0 x3 (conv tps / attn st), acc x2 (wout/ffn), a x1, dn x1, bc x1
        _PSUM_MAP = {"tps": ("mm0", ps3), "st": ("mm0", ps3),
                     "a": ("a", ps), "o": ("acc", ps2), "h": ("acc", ps2),
                     "f": ("acc", ps2), "dn": ("dn", ps), "gn1": ("dn", ps),
                     "gn2": ("dn", ps), "tpose": ("dn", ps), "bc": ("bc", ps)}

        def psum(shape, tag, dtype=F32):
            tag, pool = _PSUM_MAP[tag]
            return pool.tile(shape, dtype, tag=tag, name=tag, space="PSUM")

        # ---- constants ----
        ones_f = consts.tile([P, 1], F32, tag="ones_f", name="ones_f")
        nc.vector.memset(ones_f, 1.0)
        ones_r = consts.tile([P, 1], ADT, tag="ones_r", name="ones_r")
        nc.vector.tensor_copy(out=ones_r, in_=ones_f)
        ones1_f = consts.tile([1, P], F32, tag="ones1_f", name="ones1_f")
        nc.vector.memset(ones1_f, 1.0)
        ones1_r = consts.tile([1, P], ADT, tag="ones1_r", name="ones1_r")
        nc.vector.tensor_copy(out=ones1_r, in_=ones1_f)
        eps_t = consts.tile([1, 1], F32, tag="eps", name="eps")
        nc.vector.memset(eps_t, EPS)
        ident_f = consts.tile([P, P], F32, tag="ident_f", name="ident_f")
        from concourse.masks import make_identity
        make_identity(nc, ident_f)
        ident_r = consts.tile([P, P], ADT, tag="ident_r", name="ident_r")
        nc.vector.tensor_copy(out=ident_r, in_=ident_f)

        # ---- weights: DMA f32, round to f32r via DVE copies ----
        def load_r(dram_ap, tag, dt=None):
            st = stage.tile(list(dram_ap.shape), F32, tag="stage", name="stage")
            nc.sync.dma_start(out=st, in_=dram_ap)
            wt = wp.tile(list(dram_ap.shape), dt or ADT, tag=tag, name=tag)
            nc.vector.tensor_copy(out=wt, in_=st)
            return wt

        def load_f(dram_ap, tag):
            t = wp.tile(list(dram_ap.shape), F32, tag=tag, name=tag)
            nc.sync.dma_start(out=t, in_=dram_ap)
            return t

        win_sb = [load_r(win_d[i], f"win{i}", CDT) for i in range(54)]
        wouta_sb = [load_r(wouta_d[i], f"wouta{i}") for i in range(2)]
        woutu0_sb = [load_r(woutu0_d[i], f"woutu0{i}") for i in range(2)]
        woutu1_sb = [load_r(woutu1_d[i], f"woutu1{i}") for i in range(2)]
        w1t_sb = [[load_r(w1t_d[kt, ot], f"w1t{kt}{ot}") for ot in range(4)]
                  for kt in range(2)]
        w2t_sb = [[load_r(w2t_d[kt, ot], f"w2t{kt}{ot}") for ot in range(2)]
                  for kt in range(4)]
        bin_sb = [load_f(bin_d[i], f"bin{i}") for i in range(3)]
        bout_sb = [load_f(bout_d[i], f"bout{i}") for i in range(2)]
        b1_sb = [load_f(b1_d[i], f"b1_{i}") for i in range(4)]
        b2_sb = [load_f(b2_d[i], f"b2_{i}") for i in range(2)]
        bv_sb = load_f(bv_d, "bv")
        wk_sb = load_f(wk_d, "wk")
        wv_sb = load_f(wv_d, "wv")
        gnw_sb = [load_f(gnw_d[i], f"gnw{i}") for i in range(2)]
        gnb_sb = [load_f(gnb_d[i], f"gnb{i}") for i in range(2)]

        for s in [s for _ in range(nrep) for s in range(SPC)]:
            _build_sample(nc, tc, s, xv_d, ov_d, psum,
                          xin, scratch, tp, kvp, small, blk, blk2,
                          win_sb, bin_sb, wk_sb, wv_sb, bv_sb,
                          wouta_sb, woutu0_sb, woutu1_sb, bout_sb,
                          w1t_sb, b1_sb, w2t_sb, b2_sb, gnw_sb, gnb_sb,
                          ones_f, ones_r, ones1_f, ones1_r, eps_t, ident_r)

    nc.compile()
    return nc


def _build_sample(nc, tc, s, xv_d, ov_d, psum,
                  xin, scratch, tp, kvp, small, blk, blk2,
                  win_sb, bin_sb, wk_sb, wv_sb, bv_sb,
                  wouta_sb, woutu0_sb, woutu1_sb, bout_sb,
                  w1t_sb, b1_sb, w2t_sb, b2_sb, gnw_sb, gnb_sb,
                  ones_f, ones_r, ones1_f, ones1_r, eps_t, ident_r):
    # ============ P1: GroupNorm stats (stream x, pass 1) ============
    stats = [small.tile([P, NBLK, 6], F32, tag=f"stats{ct}", name=f"stats{ct}") for ct in range(2)]
    for ct in range(2):
        for b in range(NBLK):
            xc = xin.tile([P, BW], F32, tag="xc", name="xc")
            nc.sync.dma_start(out=xc, in_=xv_d[s, ct, :, b * BW:(b + 1) * BW])
            nc.vector.bn_stats(out=stats[ct][:, b, :], in_=xc)
    packed = small.tile([P, 4], F32, tag="packed", name="packed")
    for ct in range(2):
        mv = small.tile([P, 2], F32, tag=f"mv{ct}", name=f"mv{ct}")
        nc.vector.bn_aggr(out=mv, in_=stats[ct])
        # packed cols: [mean_ct, E[x^2]_ct]
        nc.vector.tensor_copy(out=packed[:, 2 * ct:2 * ct + 1], in_=mv[:, 0:1])
        m2 = small.tile([P, 1], F32, tag=f"m2{ct}", name=f"m2{ct}")
        nc.vector.tensor_mul(out=m2, in0=mv[:, 0:1], in1=mv[:, 0:1])
        nc.vector.tensor_add(out=packed[:, 2 * ct + 1:2 * ct + 2],
                             in0=mv[:, 1:2], in1=m2)
    gsum = psum([1, 4], "gn1")
    nc.tensor.matmul(gsum, ones_f, packed, start=True, stop=True)
    # scalars on partition 0
    gs = small.tile([1, 4], F32, tag="gs", name="gs")
    nc.vector.tensor_copy(out=gs, in_=gsum)
    sc = small.tile([1, 4], F32, tag="sc", name="sc")  # [mu, e2, var, sd]
    nc.vector.tensor_add(out=sc[:, 0:1], in0=gs[:, 0:1], in1=gs[:, 2:3])
    nc.scalar.mul(out=sc[:, 0:1], in_=sc[:, 0:1], mul=1.0 / DIM)
    nc.vector.tensor_add(out=sc[:, 1:2], in0=gs[:, 1:2], in1=gs[:, 3:4])
    nc.scalar.mul(out=sc[:, 1:2], in_=sc[:, 1:2], mul=1.0 / DIM)
    mu2 = small.tile([1, 1], F32, tag="mu2", name="mu2")
    nc.vector.tensor_mul(out=mu2, in0=sc[:, 0:1], in1=sc[:, 0:1])
    nc.vector.tensor_tensor(out=sc[:, 2:3], in0=sc[:, 1:2], in1=mu2,
                            op=OP.subtract)
    nc.scalar.activation(out=sc[:, 3:4], in_=sc[:, 2:3], func=AF.Sqrt,
                         bias=eps_t, scale=1.0)
    rv = small.tile([1, 2], F32, tag="rv", name="rv")  # [mu, rstd]
    nc.vector.tensor_copy(out=rv[:, 0:1], in_=sc[:, 0:1])
    nc.vector.reciprocal(out=rv[:, 1:2], in_=sc[:, 3:4])
    gbc = psum([P, 2], "gn2")
    nc.tensor.matmul(gbc, ones1_f, rv, start=True, stop=True)  # bcast [128,2]
    A = [small.tile([P, 1], F32, tag=f"A{ct}", name=f"A{ct}") for ct in range(2)]
    B = [small.tile([P, 1], F32, tag=f"B{ct}", name=f"B{ct}") for ct in range(2)]
    for ct in range(2):
        nc.vector.tensor_mul(out=A[ct], in0=gnw_sb[ct], in1=gbc[:, 1:2])
        tmp = small.tile([P, 1], F32, tag=f"ab{ct}", name=f"ab{ct}")
        nc.vector.tensor_mul(out=tmp, in0=A[ct], in1=gbc[:, 0:1])
        nc.vector.tensor_tensor(out=B[ct], in0=gnb_sb[ct], in1=tmp,
                                op=OP.subtract)

    # ============ P2: xn_pad (stream x, pass 2) + 3x3 conv ============
    xn = [scratch.tile([P, NPAD], CDT, tag="big", name="big") for _ in range(2)]
    xnv = [t.rearrange("p (h w) -> p h w", w=WP) for t in xn]
    for ct in range(2):
        v = xnv[ct]
        def _z(ap):
            nc.vector.memset(ap.bitcast(F32) if CDT is F32R else ap, 0.0)
        _z(v[:, 0, :])
        _z(v[:, H + 1, :])
        _z(v[:, 1:H + 1, 0:1])
        _z(v[:, 1:H + 1, WP - 1:WP])
        for b in range(NBLK):
            xc = xin.tile([P, BW], F32, tag="xc", name="xc")
            nc.sync.dma_start(out=xc, in_=xv_d[s, ct, :, b * BW:(b + 1) * BW])
            nc.vector.tensor_scalar(
                out=v[:, 1 + RB * b:1 + RB * (b + 1), 1:W + 1],
                in0=xc.rearrange("p (h w) -> p h w", w=W),
                scalar1=A[ct], scalar2=B[ct], op0=OP.mult, op1=OP.add)

    t_sb = [tp.tile([P, N], ADT, tag=f"t{ot}", name=f"t{ot}") for ot in range(3)]
    for ot in range(3):
        for b in range(NBLK):
            y0 = RB * b
            pt = psum([P, BW], "tps")
            first = True
            for ct in range(2):
                for ky in range(3):
                    for kx in range(3):
                        last = (ct == 1 and ky == 2 and kx == 2)
                        nc.tensor.matmul(
                            pt, win_sb[(ky * 3 + kx) * 2 * 3 + ct * 3 + ot],
                            xnv[ct][:, y0 + ky:y0 + ky + RB, kx:kx + W],
                            start=first, stop=last)
                        first = False
            nc.scalar.activation(
                out=t_sb[ot][:, y0 * W:(y0 + RB) * W], in_=pt,
                func=AF.Identity, bias=bin_sb[ot], scale=1.0)

    # ============ P3: k/v depthwise stride-2 downsample, vfT ============
    kf = kvp.tile([P, M], ADT, tag="kf", name="kf")   # valid on partitions 64:128
    vf = kvp.tile([P, M], ADT, tag="vf", name="vf")
    for (dst, dview, src, wsc, p0) in (
            (kf, kf[64:, :].rearrange("p (h w) -> p h w", w=32),
             t_sb[1][64:, :].rearrange("p (h w) -> p h w", w=W), wk_sb, 64),
            (vf, vf.rearrange("p (h w) -> p h w", w=32),
             t_sb[2].rearrange("p (h w) -> p h w", w=W), wv_sb, 0)):
        for j, (dy, dx) in enumerate(((0, 0), (0, 1), (1, 0), (1, 1))):
            sj = src[:, dy::2, dx::2]
            if j == 0:
                nc.vector.tensor_scalar(out=dview, in0=sj,
                                        scalar1=wsc[p0:, 0:1], scalar2=None,
                                        op0=OP.mult)
            else:
                nc.vector.scalar_tensor_tensor(
                    out=dview, in0=sj, scalar=wsc[p0:, j:j + 1], in1=dview,
                    op0=OP.mult, op1=OP.add)

    vfT = []
    for mt in range(8):
        ptr = psum([P, P], "tpose", dtype=ADT)
        nc.tensor.transpose(ptr, vf[:, mt * P:(mt + 1) * P], ident_r)
        vt = kvp.tile([P, P], ADT, tag=f"vfT{mt}", name=f"vfT{mt}")
        nc.vector.tensor_copy(out=vt, in_=ptr)
        vfT.append(vt)

    # ============ P4: attention + FFN, 8 blocks of 512 ============
    qf = t_sb[0][64:, :]
    for b in range(NBLK):
        n0 = b * BW
        est = scratch.tile([P, 8, BW], ADT, tag="big", name="big")  # exp(St) supertile
        for mt in range(8):
            st_ps = psum([P, BW], "st")
            nc.tensor.matmul(st_ps, kf[64:, mt * P:(mt + 1) * P],
                             qf[:, n0:n0 + BW], start=True, stop=True)
            nc.scalar.activation(out=est[:, mt, :], in_=st_ps, func=AF.Exp,
                                 scale=SCALE)
        # a_raw = vf @ expSt  (contraction over m)
        a_ps = psum([P, BW], "a")
        for mt in range(8):
            nc.tensor.matmul(a_ps, vfT[mt], est[:, mt, :],
                             start=(mt == 0), stop=(mt == 7))
        # denominator: sum over m (partitions) via accumulating ones-matmuls
        dn_ps = psum([1, BW], "dn")
        for mt in range(8):
            nc.tensor.matmul(dn_ps, ones_r, est[:, mt, :],
                             start=(mt == 0), stop=(mt == 7))
        rec = blk.tile([1, BW], ADT, tag="rec", name="rec")
        nc.vector.reciprocal(out=rec, in_=dn_ps)
        bc_ps = psum([P, BW], "bc")
        nc.tensor.matmul(bc_ps, ones1_r, rec, start=True, stop=True)
        bc = blk.tile([P, BW], F32, tag="bc", name="bc")
        nc.vector.tensor_copy(out=bc, in_=bc_ps)
        asc = blk.tile([P, BW], F32, tag="asc", name="asc")
        nc.vector.tensor_mul(out=asc, in0=a_ps, in1=bc)
        ra = blk.tile([P, BW], ADT, tag="ra", name="ra")
        nc.scalar.activation(out=ra, in_=asc, func=AF.Relu, bias=bv_sb,
                             scale=1.0)
        ru = [blk.tile([64, BW], ADT, tag=f"ru{i}", name=f"ru{i}") for i in range(2)]
        for i in range(2):
            nc.scalar.activation(out=ru[i], in_=t_sb[i][0:64, n0:n0 + BW],
                                 func=AF.Relu)
        # wout 1x1 + bias + residual -> x1
        x1 = []
        for ot in range(2):
            o_ps = psum([P, BW], "o")
            nc.tensor.matmul(o_ps, wouta_sb[ot], ra, start=True, stop=False)
            nc.tensor.matmul(o_ps, woutu0_sb[ot], ru[0], start=False, stop=False)
            nc.tensor.matmul(o_ps, woutu1_sb[ot], ru[1], start=False, stop=True)
            xr = xin.tile([P, BW], F32, tag="xres", name="xres")
            nc.sync.dma_start(out=xr, in_=xv_d[s, ot, :, n0:n0 + BW])
            x1t = blk2.tile([P, BW], ADT, tag=f"x1_{ot}", name=f"x1_{ot}")
            nc.vector.scalar_tensor_tensor(out=x1t, in0=o_ps,
                                           scalar=bout_sb[ot], in1=xr,
                                           op0=OP.add, op1=OP.add)
            x1.append(x1t)
        # FFN
        hs = []
        for ot in range(4):
            h_ps = psum([P, BW], "h")
            for kt in range(2):
                nc.tensor.matmul(h_ps, w1t_sb[kt][ot], x1[kt],
                                 start=(kt == 0), stop=(kt == 1))
            ht = blk.tile([P, BW], ADT, tag=f"h{ot}", name=f"h{ot}")
            nc.scalar.activation(out=ht, in_=h_ps, func=AF.Relu,
                                 bias=b1_sb[ot], scale=1.0)
            hs.append(ht)
        for ot in range(2):
            f_ps = psum([P, BW], "f")
            for kt in range(4):
                nc.tensor.matmul(f_ps, w2t_sb[kt][ot], hs[kt],
                                 start=(kt == 0), stop=(kt == 3))
            ob = blk2.tile([P, BW], F32, tag=f"ob{ot}", name=f"ob{ot}")
            nc.vector.scalar_tensor_tensor(out=ob, in0=f_ps,
                                           scalar=b2_sb[ot], in1=x1[ot],
                                           op0=OP.add, op1=OP.add)
            nc.sync.dma_start(out=ov_d[s, ot, :, n0:n0 + BW], in_=ob)


def kernel(**inputs):
    x = np.ascontiguousarray(np.asarray(inputs["x"], dtype=np.float32))
    B = x.shape[0]
    assert B == NCORES * SPC
    w = _prep_weights(
        inputs["w_in"], inputs["s_in"], inputs["b_in"],
        inputs["w_k"], inputs["s_k"], inputs["w_v"], inputs["s_v"],
        inputs["b_v"], inputs["w_out"], inputs["s_out"], inputs["b_out"],
        inputs["w1"], inputs["s1"], inputs["b1"],
        inputs["w2"], inputs["s2"], inputs["b2"],
        inputs["gn_w"], inputs["gn_b"])
    w = {k: np.ascontiguousarray(v) for k, v in w.items()}

    if "nc" not in _CACHE:
        _CACHE["nc"] = _build(NCORES)
    nc = _CACHE["nc"]

    in_maps = []
    for c in range(NCORES):
        m = dict(w)
        m["x"] = np.ascontiguousarray(
            x[c * SPC:(c + 1) * SPC].reshape(SPC, DIM, N))
        in_maps.append(m)

    res = run_bass_kernel_spmd(nc, in_maps, list(range(NCORES)))
    _CACHE["last_result"] = res
    out = np.concatenate([r["out"] for r in res.results], axis=0)
    return out.reshape(B, DIM, H, W).astype(np.float32)


if __name__ == "__main__":
    rng = np.random.default_rng(0)
    # smoke test with random weights only (no reference available here)
    print("building...")
    nc = _build(NCORES)
    print("built ok")
